# revision 1
# baseline (speedup 1.0000x reference)
"""Trainium2 Bass kernel for the pose-estimation loss (pm / t_center / t_depth).

Strategy
--------
pm[n] = mean_p | (pred_R[n]-gt_R[n]) @ obj_points[obj_id[n], p] |_1 / diam[obj_id[n]]

The data-dependent gather obj_points[obj_id] is folded into the matmul:
    Y[(i,n), p] = sum_{o,j} A[(o,j),(i,n)] * B[(o,j), p]
with A[(o,j),(i,n)] = [obj_id[n]==o] * dR[n,i,j]   (24 x 384, built on host)
     B[(o,j), p]    = obj_points[o, p, j]          (24 x 100000)
The one-hot selection is free on the tensor engine (contraction K=24 < 128).

Sharding: points are split across the 8 cores (12500 each).  Within a core the
12500 points are split into 4 row-groups of 3136 (7 chunks x 448, zero padded)
living at SBUF partitions 32g..32g+23, so that 4 matmuls run concurrently in
distinct PE row-groups and the B DMA touches 96 partitions.

Each matmul produces PSUM [128, 448] = (samples for coord i) x (points).  A
fused abs+sum-over-free-dim drains a whole 4-bank PSUM tile at once, using the
ScalarE activation(Abs, accum_out=...) and VectorE tensor_reduce(
apply_absolute_value=True) alternately so both engines share the load.

Per core output: out[128, 3] = [pm partial sum, t_center, t_depth].
Host: pm = sum_over_cores(out[:,0]) / 100000 / diam[obj_id].
"""

import os
import sys

import numpy as np

os.environ.setdefault("MYCRO_LOCAL_CACHE", "1")
if "/opt/trn_rl_repo" not in sys.path:
    sys.path.insert(0, "/opt/trn_rl_repo")

# ---- problem constants (hardcoded, must match the reference) ----
N_SAMPLES = 128
NUM_OBJECTS = 8
NUM_POINTS = 100000
N_CORES = 8

PTS_PER_CORE = NUM_POINTS // N_CORES  # 12500
GROUPS = 4                            # PE row-groups per core
CHUNK = 448                           # matmul moving free dim (<=512)
QCHUNKS = 7                           # chunks per group
GROUP_PTS = CHUNK * QCHUNKS           # 3136 point slots per group (3092 real in g=3)
ICHUNKS = 3                           # sample-coord chunks: 384 = 3 * 128
N_ACC = QCHUNKS * ICHUNKS * 2         # accumulator columns (2 halves per (q,i))

_CACHE = {}


def _build_module():
    """Build + compile the single-core Bass program (same program on all cores)."""
    if "nc" in _CACHE:
        return _CACHE["nc"]

    from contextlib import ExitStack

    import concourse.bass as bass  # noqa: F401  (import registers engines)
    import concourse.tile as tile
    from concourse import bacc, mybir

    f32 = mybir.dt.float32
    bf16 = mybir.dt.bfloat16

    nc = bacc.Bacc("TRN2", target_bir_lowering=False, debug=False)

    A_COLS = ICHUNKS * 128            # 384
    AB_COLS = A_COLS + GROUP_PTS      # amat columns then B columns
    abmat = nc.dram_tensor("abmat", [128, AB_COLS], bf16, kind="ExternalInput").ap()
    tsite = nc.dram_tensor("tsite", [128, 6], f32, kind="ExternalInput").ap()
    out = nc.dram_tensor("out", [128, 3], f32, kind="ExternalOutput").ap()

    with ExitStack() as ctx:
        tc = ctx.enter_context(tile.TileContext(nc))
        const = ctx.enter_context(tc.tile_pool(name="const", bufs=1))
        # 2-bank PSUM tiles x 4 bufs: while ScalarE+VectorE each drain one
        # tile, the PE still has two free tiles to fill (no reducer gaps).
        psum = ctx.enter_context(tc.tile_pool(name="psum", bufs=4, space="PSUM"))

        ab_sb = const.tile([128, AB_COLS], bf16)
        a_sb = ab_sb[:, 0:A_COLS]
        ts_sb = const.tile([128, 6], f32)
        acc = const.tile([128, N_ACC], f32)
        dummy = const.tile([128, 2, CHUNK], f32)
        out_sb = const.tile([128, 3], f32)
        warm = const.tile([128, 1], f32)
        wmm = const.tile([128, 640], bf16)
        d_sb = const.tile([128, 3], f32)

        # Warm up the ACT table set (Abs) so the ~2.7us table load overlaps DMA.
        nc.vector.memset(warm, 0.0)
        nc.scalar.activation(out=warm, in_=warm, func=mybir.ActivationFunctionType.Abs)

        # DMA: A + first B chunk in one issue so the first matmuls start early,
        # the rest of B in 2 blocks; tsite from gpsimd (tiny, off sync's queue).
        nc.sync.dma_start(out=ab_sb[:, 0 : A_COLS + CHUNK],
                          in_=abmat[:, 0 : A_COLS + CHUNK])
        nc.sync.dma_start(out=ab_sb[:, A_COLS + CHUNK : A_COLS + 4 * CHUNK],
                          in_=abmat[:, A_COLS + CHUNK : A_COLS + 4 * CHUNK])
        nc.sync.dma_start(out=ab_sb[:, A_COLS + 4 * CHUNK :],
                          in_=abmat[:, A_COLS + 4 * CHUNK :])
        nc.gpsimd.dma_start(out=ts_sb, in_=tsite)

        # HAM warm-up: ~3.6us of dummy matmuls on zeros while the DMAs land,
        # so the real matmuls run at 2.4 GHz instead of the cold 1.2 GHz.
        nc.vector.memset(wmm, 0.0)
        for _ in range(10):
            wps = psum.tile([128, 2, 512], f32, tag="ps")
            nc.tensor.matmul(
                wps[:, 0, :], lhsT=wmm[0:24, 0:128], rhs=wmm[0:24, 128:640],
                start=True, stop=True,
            )

        # t_site losses (tiny): d = gt - pred; t_center = |d0|+|d1|; t_depth = |d2|
        nc.vector.tensor_sub(d_sb, ts_sb[:, 0:3], ts_sb[:, 3:6])
        nc.vector.tensor_reduce(
            out=out_sb[:, 1:2], in_=d_sb[:, 0:2], axis=mybir.AxisListType.X,
            op=mybir.AluOpType.add, apply_absolute_value=True,
        )
        nc.vector.tensor_reduce(
            out=out_sb[:, 2:3], in_=d_sb[:, 2:3], axis=mybir.AxisListType.X,
            op=mybir.AluOpType.add, apply_absolute_value=True,
        )

        col = 0
        for q in range(QCHUNKS):
            for i in range(ICHUNKS):
                for half in range(2):
                    ps = psum.tile([128, 2, 512], f32)
                    for gg in range(2):
                        g = 2 * half + gg
                        nc.tensor.matmul(
                            ps[:, gg, 0:CHUNK],
                            lhsT=a_sb[32 * g : 32 * g + 24, i * 128 : (i + 1) * 128],
                            rhs=ab_sb[32 * g : 32 * g + 24,
                                      A_COLS + q * CHUNK : A_COLS + (q + 1) * CHUNK],
                            start=True,
                            stop=True,
                            tile_position=(32 * g, 0),
                        )
                    red_in = ps[:, :, 0:CHUNK]
                    if col % 2 == 0:
                        nc.scalar.activation(
                            out=dummy,
                            in_=red_in,
                            func=mybir.ActivationFunctionType.Abs,
                            accum_out=acc[:, col : col + 1],
                        )
                    else:
                        nc.vector.tensor_reduce(
                            out=acc[:, col : col + 1],
                            in_=red_in,
                            axis=mybir.AxisListType.XY,
                            op=mybir.AluOpType.add,
                            apply_absolute_value=True,
                        )
                    col += 1

        nc.vector.tensor_reduce(
            out=out_sb[:, 0:1], in_=acc, axis=mybir.AxisListType.X,
            op=mybir.AluOpType.add,
        )
        nc.sync.dma_start(out=out, in_=out_sb)

    nc.compile()
    _CACHE["nc"] = nc
    return nc


def _prepare_in_maps(obj_id, gt_cam_R_m2c, pred_cam_R_m2c, gt_cam_t_m2c_site,
                     pred_cam_t_m2c_site, obj_points, obj_diameters):
    obj_id = np.asarray(obj_id).astype(np.int64)
    dR = (np.asarray(pred_cam_R_m2c, np.float32)
          - np.asarray(gt_cam_R_m2c, np.float32))          # [N, 3, 3] (i, j)
    pts = np.asarray(obj_points, np.float32)               # [8, P, 3]

    import ml_dtypes

    # A[(o,j), (i,n)] = [obj_id[n]==o] * dR[n, i, j]
    afull = np.zeros((NUM_OBJECTS, 3, 3, N_SAMPLES), np.float32)  # [o, j, i, n]
    afull[obj_id, :, :, np.arange(N_SAMPLES)] = dR.transpose(0, 2, 1)  # [n, j, i]
    a24 = afull.reshape(NUM_OBJECTS * 3, 3 * N_SAMPLES)    # rows (o,j), cols i*128+n
    a_host = np.zeros((128, ICHUNKS * 128), np.float32)
    for g in range(GROUPS):
        a_host[32 * g : 32 * g + 24] = a24

    # B rows (o,j), cols p
    b24 = pts.transpose(0, 2, 1).reshape(NUM_OBJECTS * 3, NUM_POINTS)

    ts_host = np.concatenate(
        [np.asarray(gt_cam_t_m2c_site, np.float32),
         np.asarray(pred_cam_t_m2c_site, np.float32)], axis=1)  # [128, 6]

    in_maps = []
    for c in range(N_CORES):
        bc = np.zeros((128, GROUP_PTS), np.float32)
        base = c * PTS_PER_CORE
        for g in range(GROUPS):
            s = base + GROUP_PTS * g
            e = min(base + PTS_PER_CORE, s + GROUP_PTS)
            if e > s:
                bc[32 * g : 32 * g + 24, : e - s] = b24[:, s:e]
        ab = np.ascontiguousarray(
            np.concatenate([a_host, bc], axis=1)).astype(ml_dtypes.bfloat16)
        in_maps.append({"abmat": ab, "tsite": ts_host})
    return in_maps, obj_id, np.asarray(obj_diameters, np.float32)


def _postprocess(results, obj_id, obj_diameters):
    pm_sum = np.zeros(N_SAMPLES, np.float64)
    for c in range(N_CORES):
        pm_sum += results[c]["out"][:, 0].astype(np.float64)
    pm = (pm_sum / NUM_POINTS / obj_diameters[obj_id].astype(np.float64)).astype(
        np.float32)
    t_center = results[0]["out"][:, 1].astype(np.float32)
    t_depth = results[0]["out"][:, 2].astype(np.float32)
    return pm, t_center, t_depth


def run(inputs, trace=False):
    """Run on the 8 NeuronCores. Returns ((pm, t_center, t_depth), BassKernelResults)."""
    from concourse.bass_utils import run_bass_kernel_spmd

    nc = _build_module()
    in_maps, obj_id, diam = _prepare_in_maps(**inputs)
    res = run_bass_kernel_spmd(nc, in_maps, list(range(N_CORES)), trace=trace)
    return _postprocess(res.results, obj_id, diam), res


def run_sim(inputs):
    """CoreSim path (numerics check without hardware)."""
    from concourse.bass_interp import CoreSim

    nc = _build_module()
    in_maps, obj_id, diam = _prepare_in_maps(**inputs)
    results = []
    for c in range(N_CORES):
        sim = CoreSim(nc)
        for name, val in in_maps[c].items():
            sim.tensor(name)[:] = val
        sim.simulate(check_with_hw=False)
        results.append({"out": np.array(sim.tensor("out"))})
    return _postprocess(results, obj_id, diam)


def kernel(**inputs):
    (pm, t_center, t_depth), _ = run(inputs, trace=False)
    return pm, t_center, t_depth

